# revision 23
# baseline (speedup 1.0000x reference)
"""GAT+LSTM fused kernel for 8 trn2 NeuronCores.

Key structure (per core, fully collective-free):
- The reference output depends only on batch row T-1=11 of its LSTM, so
  only GAT outputs for live nodes [110000, 120000) are needed.
- Live nodes split into 80 buckets of 128 by dst>>7. Core k owns buckets
  [10k-1, 10k+10): its 1280 output nodes PLUS the bucket containing its
  96-step LSTM warmup window (recomputed redundantly, so no cross-core
  exchange is needed anywhere; the host concatenates per-core outputs).
- Edges partitioned by dst bucket, 128-edge chunks. Per chunk:
  indirect-gather x[src] rows (bf16, 128B), PE pair-transpose, h|a_src
  via one bf16 matmul against [W_gat | W_gat@A_src | W_gat@A_dst],
  a_dst via a host-shipped transposed one-hot matmul, segment softmax
  without max subtraction (exp via sigmoid(x)/sigmoid(-x); the Exp ACT
  table is not resident), one-hot scatter matmul accumulating [dst,132]
  in PSUM. Multi-offset indirect DMAs corrupt data at these row sizes
  (verified empirically), hence one 128-offset gather per chunk.
- Self-loops handled densely from the core's own node block (no gather);
  their edge_attr is mean(edge_attr), reduced locally from the
  replicated full edge_attr - no AllReduce.
- LSTM: one chained [32, 1376] sequence (1280 cols + 96 warmup); the
  c-scan chains across the halo boundary, which the warmup absorbs
  (state memory here is only a few dozen steps; f-gates ~ sigmoid(small)).
  Fixed point, ITERS=4 (iteration error ~5e-4, far under the bf16 noise
  floor ~8e-3; tolerance 2e-2). gx lands in PSUM via a PE identity
  matmul so the Whh matmul accumulates onto it in one accumulation
  group (a cross-engine vector preload raced and gave wrong results).
- FC over the 1280 main columns; host concatenates the 8 slices.
"""
import os
import numpy as np
import ml_dtypes

import concourse.bass as bass
import concourse.bacc as bacc
import concourse.tile as tile
from concourse import mybir
from concourse.bass_utils import run_bass_kernel_spmd
from concourse.masks import make_identity
from contextlib import ExitStack

dt = mybir.dt
F32 = dt.float32
BF16 = dt.bfloat16
AF = mybir.ActivationFunctionType
ALU = mybir.AluOpType

T, N, F_IN = 12, 10000, 64
HEADS, C, HID = 4, 32, 32
E, TN = 1_000_000, 120_000
NCORES = 8
D0 = (T - 1) * N
DN = N
NB = 11                      # buckets per core (1 halo + 10 own)
NW = 96                      # LSTM warmup steps
SEQ = 1280                   # sequence cols owned per core
NTL = SEQ + NW               # 1376: one chained sequence incl warmup
ITERS = 4
LEAK = 0.2
XPAD = TN + 64               # x table rows (pad rows are zero)
EAC = (E + 127) // 128       # 7813 cols for the local edge_attr reduce

_CACHE = {}


def _prep_host(inputs):
    x = np.asarray(inputs["x_seq"], np.float32).reshape(TN, F_IN)
    ei = np.asarray(inputs["edge_index"])
    ea = np.asarray(inputs["edge_attr"], np.float32)[:, 0]
    W_gat = np.asarray(inputs["W_gat"], np.float32)
    att_src = np.asarray(inputs["att_src"], np.float32)
    att_dst = np.asarray(inputs["att_dst"], np.float32)
    att_edge = np.asarray(inputs["att_edge"], np.float32)
    W_edge = np.asarray(inputs["W_edge"], np.float32)
    gat_bias = np.asarray(inputs["gat_bias"], np.float32)
    W_ih = np.asarray(inputs["W_ih"], np.float32)
    W_hh = np.asarray(inputs["W_hh"], np.float32)
    b = np.asarray(inputs["b_ih"], np.float32) + np.asarray(inputs["b_hh"], np.float32)
    W_fc = np.asarray(inputs["W_fc"], np.float32)
    b_fc = np.asarray(inputs["b_fc"], np.float32)

    # W_all: [64, 136] = [W_gat | W_gat@A_src | W_gat@A_dst]
    A_src = np.zeros((HEADS * C, HEADS), np.float32)
    A_dst = np.zeros((HEADS * C, HEADS), np.float32)
    for h in range(HEADS):
        A_src[h * C:(h + 1) * C, h] = att_src[h]
        A_dst[h * C:(h + 1) * C, h] = att_dst[h]
    W_all = np.concatenate([W_gat, W_gat @ A_src, W_gat @ A_dst], axis=1)
    kap = np.array([np.dot(W_edge[0, h * C:(h + 1) * C], att_edge[h])
                    for h in range(HEADS)], np.float32)
    kap_rep = np.broadcast_to(kap, (128, HEADS)).copy()
    gb_row = np.broadcast_to(gat_bias, (128, HEADS * C)).copy()
    iota128 = np.broadcast_to(np.arange(128, dtype=np.float32), (128, 128)).copy()
    # gate row order [f, i, o, g] (torch order is i,f,g,o)
    perm = np.concatenate([np.arange(32, 64), np.arange(0, 32),
                           np.arange(96, 128), np.arange(64, 96)])
    WihT = np.ascontiguousarray(W_ih[perm].T)
    WhhT = np.ascontiguousarray(W_hh[perm].T)
    br = np.ascontiguousarray(b[perm].reshape(128, 1))

    xbf = np.zeros((XPAD, F_IN), ml_dtypes.bfloat16)
    xbf[:TN] = x
    eaF = np.zeros((128, EAC), np.float32)
    j = np.arange(E)
    eaF[j % 128, j // 128] = ea

    src = ei[0].astype(np.int64)
    dst = ei[1].astype(np.int64)
    live = (dst >= D0) & (dst < D0 + DN)
    sl = src[live]
    dl = dst[live] - D0
    bkt = dl >> 7
    cnt = np.bincount(bkt, minlength=80)
    chb = int(max(1, -(-int(cnt.max()) // 128)))   # chunks per bucket
    nch = NB * chb

    by_bucket = {}
    order = np.argsort(bkt, kind="stable")
    pos = 0
    for gb in range(80):
        by_bucket[gb] = order[pos:pos + cnt[gb]]
        pos += cnt[gb]

    eal = ea[live]
    in_maps = []
    for k in range(NCORES):
        xI = np.full((128, nch), TN, np.int32)
        eaC = np.zeros((128, nch), np.float32)
        dstF = np.full((128, nch), -1.0, np.float32)
        ohT = np.zeros((128, nch * 128), ml_dtypes.bfloat16)
        for b_ in range(NB):
            gb = 10 * k - 1 + b_
            if not (0 <= gb < 80):
                continue
            sel = by_bucket[gb]
            ne = len(sel)
            assert ne <= chb * 128
            ii = np.arange(ne)
            cc = b_ * chb + ii // 128
            pp = ii % 128
            xI[pp, cc] = sl[sel].astype(np.int32)
            eaC[pp, cc] = eal[sel]
            dpos = (dl[sel] - 128 * gb).astype(np.int32)
            dstF[pp, cc] = dpos.astype(np.float32)
            ohT[dpos, cc * 128 + pp] = 1.0
        # own node features, transposed: bucket-local nodes [128*(10k-1), +1408)
        xTD = np.zeros((F_IN, NB * 128), ml_dtypes.bfloat16)
        lo = 128 * (10 * k - 1)
        for j2 in range(NB * 128):
            gn = lo + j2
            if 0 <= gn < DN:
                xTD[:, j2] = x[D0 + gn]
        Bwarm = np.zeros((128, NW), np.float32)
        if k == 0:
            Bwarm[32:96, :] = -30.0    # i and o gate rows of the junk warmup
        in_maps.append({
            "xnd": xbf, "eaF": eaF,
            "xI": xI, "ohT": ohT, "eaC": eaC, "dstF": dstF,
            "xTD": np.ascontiguousarray(xTD),
            "Bwarm": Bwarm.astype(ml_dtypes.bfloat16),
            "Wall": W_all.astype(ml_dtypes.bfloat16),
            "kap": kap_rep, "gbrow": gb_row, "iota": iota128,
            "Wih": WihT.astype(ml_dtypes.bfloat16),
            "Whh": WhhT.astype(ml_dtypes.bfloat16),
            "br": br,
            "Wfc": np.ascontiguousarray(W_fc.reshape(HID, 1)).astype(ml_dtypes.bfloat16),
            "bfc": np.ascontiguousarray(b_fc.reshape(1, 1)),
        })
    return in_maps, chb


def _build_nc(chb):
    nch = NB * chb
    nc = bacc.Bacc("TRN2", target_bir_lowering=False, debug=False,
                   num_devices=NCORES)
    g = lambda n, s, d=F32: nc.dram_tensor(n, s, d, kind="ExternalInput").ap()
    xnd = g("xnd", [XPAD, F_IN], BF16)
    xI = g("xI", [128, nch], dt.int32)
    eaC = g("eaC", [128, nch])
    dstF = g("dstF", [128, nch])
    xTD = g("xTD", [F_IN, NB * 128], BF16)
    eaF = g("eaF", [128, EAC])
    ohT = g("ohT", [128, nch * 128], BF16)
    Bwarm = g("Bwarm", [128, NW], BF16)
    Wall = g("Wall", [F_IN, 136], BF16)
    kap = g("kap", [128, HEADS])
    gbrow = g("gbrow", [128, 128])
    iota = g("iota", [128, 128])
    Wih = g("Wih", [128, 128], BF16)
    Whh = g("Whh", [HID, 128], BF16)
    br = g("br", [128, 1])
    Wfc = g("Wfc", [HID, 1], BF16)
    bfc = g("bfc", [1, 1])
    out = nc.dram_tensor("out", [1, SEQ], F32, kind="ExternalOutput").ap()

    with tile.TileContext(nc) as tc, ExitStack() as top:
        const = top.enter_context(tc.tile_pool(name="const", bufs=1))
        identB = const.tile([128, 128], BF16)
        make_identity(nc, identB[:])
        wall_t = const.tile([F_IN, 136], BF16); nc.sync.dma_start(wall_t[:], Wall[:])
        kap_t = const.tile([128, HEADS], F32); nc.sync.dma_start(kap_t[:], kap[:])
        gbr_t = const.tile([128, 128], F32); nc.sync.dma_start(gbr_t[:], gbrow[:])
        iota_t = const.tile([128, 128], F32); nc.sync.dma_start(iota_t[:], iota[:])
        wih_t = const.tile([128, 128], BF16); nc.sync.dma_start(wih_t[:], Wih[:])
        whh_t = const.tile([HID, 128], BF16); nc.sync.dma_start(whh_t[:], Whh[:])
        br_t = const.tile([128, 1], F32); nc.sync.dma_start(br_t[:], br[:])
        wfc_t = const.tile([HID, 1], BF16); nc.sync.dma_start(wfc_t[:], Wfc[:])
        bfc_t = const.tile([1, 1], F32); nc.sync.dma_start(bfc_t[:], bfc[:])
        bw_t = const.tile([128, NW], BF16); nc.sync.dma_start(bw_t[:], Bwarm[:])
        xi_t = const.tile([128, nch], dt.int32); nc.sync.dma_start(xi_t[:], xI[:])
        ohT_t = const.tile([128, nch * 128], BF16); nc.sync.dma_start(ohT_t[:], ohT[:])
        eac_t = const.tile([128, nch], F32); nc.sync.dma_start(eac_t[:], eaC[:])
        dsf_t = const.tile([128, nch], F32); nc.sync.dma_start(dsf_t[:], dstF[:])
        xtd_t = const.tile([F_IN, NB * 128], BF16); nc.sync.dma_start(xtd_t[:], xTD[:])
        meanr = const.tile([128, 1], F32)
        gatT = const.tile([128, NB * 128], BF16)     # [feat, bucket-local node]

        # ---------- Phase 0: mean(edge_attr), local full reduce ----------
        with ExitStack() as ph:
            sbm = ph.enter_context(tc.tile_pool(name="sbm", bufs=1))
            psm = ph.enter_context(tc.tile_pool(name="psm", bufs=1, space="PSUM"))
            eaf_t = sbm.tile([128, EAC], F32)
            nc.sync.dma_start(eaf_t[:], eaF[:])
            eap = sbm.tile([128, 1], F32)
            nc.vector.tensor_reduce(eap[:], eaf_t[:], mybir.AxisListType.X, ALU.add)
            onc = sbm.tile([128, 1], F32)
            nc.gpsimd.memset(onc[:], 1.0)
            ps1 = psm.tile([1, 1], F32, space="PSUM", tag="ps1")
            nc.tensor.matmul(ps1[:], lhsT=eap[:], rhs=onc[:], start=True, stop=True)
            eas = sbm.tile([1, 1], F32)
            nc.scalar.mul(eas[:], ps1[:], 1.0 / E)
            onr = sbm.tile([1, 128], F32)
            nc.gpsimd.memset(onr[:], 1.0)
            ps2 = psm.tile([128, 1], F32, space="PSUM", tag="ps2")
            nc.tensor.matmul(ps2[:], lhsT=onr[:], rhs=eas[:], start=True, stop=True)
            nc.vector.tensor_copy(meanr[:], ps2[:])

        # ---------- Phase 1: self tables (h|a_src|a_dst for own nodes) ----
        sfp = top.enter_context(tc.tile_pool(name="sfp", bufs=1))
        SF = sfp.tile([128, NB * 136], F32)
        SFv = SF[:].rearrange("p (j w) -> p j w", w=136)
        adB = sfp.tile([128, NB * 4], BF16)
        adBv = adB[:].rearrange("p (j w) -> p j w", w=4)
        selfSC = sfp.tile([128, NB * 132], BF16)
        sSCv = selfSC[:].rearrange("p (j w) -> p j w", w=132)
        with ExitStack() as ph:
            sbs = ph.enter_context(tc.tile_pool(name="sbs", bufs=1))
            pss = ph.enter_context(tc.tile_pool(name="pss", bufs=4, space="PSUM"))
            for b_ in range(NB):
                pf = pss.tile([128, 136], F32, space="PSUM", tag="pf")
                nc.tensor.matmul(pf[:], lhsT=xtd_t[:, b_ * 128:(b_ + 1) * 128],
                                 rhs=wall_t[:], start=True, stop=True)
                nc.vector.tensor_copy(SFv[:, b_, :], pf[:])
            nc.vector.tensor_copy(adBv, SFv[:, :, 132:136])
            QS = sbs.tile([128, NB * 4], F32)
            QSv = QS[:].rearrange("p (j w) -> p j w", w=4)
            nc.vector.tensor_tensor(out=QSv, in0=SFv[:, :, 128:132],
                                    in1=SFv[:, :, 132:136], op=ALU.add)
            kapb = kap_t[:].rearrange("p (o w) -> p o w", o=1) \
                .to_broadcast([128, NB, 4])
            nc.vector.scalar_tensor_tensor(out=QSv, in0=kapb, scalar=meanr[:],
                                           op0=ALU.mult, op1=ALU.add, in1=QSv)
            T2 = sbs.tile([128, NB * 4], F32)
            nc.vector.tensor_scalar_mul(T2[:], QS[:], LEAK)
            nc.vector.tensor_tensor(out=QS[:], in0=QS[:], in1=T2[:], op=ALU.max)
            SG1 = sbs.tile([128, NB * 4], F32)
            nc.scalar.activation(SG1[:], QS[:], AF.Sigmoid)
            nc.scalar.activation(T2[:], QS[:], AF.Sigmoid, scale=-1.0)
            nc.vector.reciprocal(T2[:], T2[:])
            SSf = sbs.tile([128, NB * 4], F32)
            nc.vector.tensor_tensor(out=SSf[:], in0=SG1[:], in1=T2[:], op=ALU.mult)
            ssf4 = SSf[:].rearrange("p (j h w) -> p j h w", h=HEADS, w=1) \
                .to_broadcast([128, NB, HEADS, C])
            sf4 = SFv[:, :, 0:128].rearrange("p j (h c) -> p j h c", h=HEADS)
            o4 = sSCv[:, :, 0:128].rearrange("p j (h c) -> p j h c", h=HEADS)
            nc.vector.tensor_tensor(out=o4, in0=sf4, in1=ssf4, op=ALU.mult)
            nc.vector.tensor_copy(sSCv[:, :, 128:132],
                                  SSf[:].rearrange("p (j w) -> p j w", w=4))

        # ---------- Phase 2: edge phase ----------
        NXT = (chb + 1) // 2          # transpose pair count per bucket
        with ExitStack() as ph:
            sbe = ph.enter_context(tc.tile_pool(name="sbe", bufs=2))
            sbq = ph.enter_context(tc.tile_pool(name="sbq", bufs=2))
            pse = ph.enter_context(tc.tile_pool(name="pse", bufs=2, space="PSUM"))
            psh = ph.enter_context(tc.tile_pool(name="psh", bufs=2, space="PSUM"))
            psa = ph.enter_context(tc.tile_pool(name="psa", bufs=2, space="PSUM"))
            psk = ph.enter_context(tc.tile_pool(name="psk", bufs=2, space="PSUM"))
            for b_ in range(NB):
                c0 = b_ * chb
                XGBt = sbe.tile([128, chb * 64], BF16, tag="XGB")
                XGBv = XGBt[:].rearrange("p (e w) -> p e w", w=64)
                for cc in range(chb):
                    nc.gpsimd.indirect_dma_start(
                        out=XGBv[:, cc, :], out_offset=None, in_=xnd[:],
                        in_offset=bass.IndirectOffsetOnAxis(
                            ap=xi_t[:, c0 + cc:c0 + cc + 1], axis=0))
                XGB = XGBt[:]
                XT = sbe.tile([F_IN, chb * 128], BF16, tag="XT")
                for pr in range(NXT):
                    w = min(128, chb * 64 - pr * 128)
                    pxT = psk.tile([128, 128], BF16, space="PSUM", tag="ptr")
                    nc.tensor.transpose(out=pxT[0:w, :],
                                        in_=XGB[:, pr * 128:pr * 128 + w],
                                        identity=identB[:])
                    nc.vector.tensor_copy(XT[:, (2 * pr) * 128:(2 * pr + 1) * 128],
                                          pxT[0:64, :])
                    if w > 64:
                        nc.vector.tensor_copy(
                            XT[:, (2 * pr + 1) * 128:(2 * pr + 2) * 128],
                            pxT[64:128, :])
                # h|a_src per edge; batches of 3 chunks share one PSUM tile
                hEs = sbe.tile([128, chb * 132], BF16, tag="hEs")
                hEv = hEs[:].rearrange("p (e w) -> p e w", w=132)
                Qb = sbq.tile([128, chb * 4], F32, tag="Qb")
                Qv = Qb[:].rearrange("p (e w) -> p e w", w=4)
                nb3 = (chb + 2) // 3
                for b3 in range(nb3):
                    n3 = min(3, chb - b3 * 3)
                    phE = psh.tile([128, 3 * 132], F32, space="PSUM", tag="phE")
                    for j3 in range(n3):
                        cc = b3 * 3 + j3
                        lhs = XT[:, cc * 128:(cc + 1) * 128]
                        nc.tensor.matmul(phE[:, j3 * 132:(j3 + 1) * 132],
                                         lhsT=lhs, rhs=wall_t[:, 0:132],
                                         start=True, stop=True)
                    phEv = phE[:].rearrange("p (e w) -> p e w", w=132)
                    nc.vector.tensor_copy(hEv[:, b3 * 3:b3 * 3 + n3, :],
                                          phEv[:, 0:n3, :])
                    nc.vector.tensor_copy(Qv[:, b3 * 3:b3 * 3 + n3, :],
                                          phEv[:, 0:n3, 128:132])
                # one-hots for the whole bucket in one op
                ohs = sbe.tile([128, chb * 128], BF16, tag="ohs")
                ohv = ohs[:].rearrange("p (e w) -> p e w", w=128)
                dfb = dsf_t[:, c0:c0 + chb].rearrange("p (e w) -> p e w", w=1) \
                    .to_broadcast([128, chb, 128])
                iob = iota_t[:].rearrange("p (o w) -> p o w", o=1) \
                    .to_broadcast([128, chb, 128])
                nc.vector.tensor_tensor(out=ohv, in0=dfb, in1=iob, op=ALU.is_equal)
                # a_dst per edge: one-hot-transpose (host-shipped) matmul
                padc = psa.tile([128, chb * 4], F32, space="PSUM", tag="padc")
                for cc in range(chb):
                    nc.tensor.matmul(
                        padc[:, cc * 4:(cc + 1) * 4],
                        lhsT=ohT_t[:, (c0 + cc) * 128:(c0 + cc + 1) * 128],
                        rhs=adBv[:, b_, :], start=True, stop=True)
                # q = a_src + a_dst + ea*kap ; s = exp(leaky_relu(q))
                kmb = sbq.tile([128, chb * 4], F32, tag="kmb")
                kmv = kmb[:].rearrange("p (e w) -> p e w", w=4)
                eab = eac_t[:, c0:c0 + chb].rearrange("p (e w) -> p e w", w=1) \
                    .to_broadcast([128, chb, 4])
                kab = kap_t[:].rearrange("p (o w) -> p o w", o=1) \
                    .to_broadcast([128, chb, 4])
                nc.vector.tensor_tensor(out=kmv, in0=eab, in1=kab, op=ALU.mult)
                nc.vector.tensor_tensor(out=Qb[:], in0=Qb[:], in1=kmb[:], op=ALU.add)
                nc.vector.tensor_tensor(out=Qb[:], in0=Qb[:], in1=padc[:],
                                        op=ALU.add)
                nc.vector.tensor_scalar_mul(kmb[:], Qb[:], LEAK)
                nc.vector.tensor_tensor(out=Qb[:], in0=Qb[:], in1=kmb[:], op=ALU.max)
                SG = sbq.tile([128, chb * 4], F32, tag="SG")
                nc.scalar.activation(SG[:], Qb[:], AF.Sigmoid)
                nc.scalar.activation(kmb[:], Qb[:], AF.Sigmoid, scale=-1.0)
                nc.vector.reciprocal(kmb[:], kmb[:])
                Sbf = sbq.tile([128, chb * 4], BF16, tag="Sbf")
                nc.vector.tensor_tensor(out=Sbf[:], in0=SG[:], in1=kmb[:], op=ALU.mult)
                Sbv = Sbf[:].rearrange("p (e w) -> p e w", w=4)
                # messages and scatter
                SCb = sbe.tile([128, chb * 132], BF16, tag="SCb")
                SCv = SCb[:].rearrange("p (e w) -> p e w", w=132)
                sb4 = Sbf[:].rearrange("p (e h w) -> p e h w", h=HEADS, w=1) \
                    .to_broadcast([128, chb, HEADS, C])
                he4 = hEv[:, :, 0:128].rearrange("p e (h c) -> p e h c", h=HEADS)
                sc4 = SCv[:, :, 0:128].rearrange("p e (h c) -> p e h c", h=HEADS)
                nc.vector.tensor_tensor(out=sc4, in0=he4, in1=sb4, op=ALU.mult)
                nc.vector.tensor_copy(SCv[:, :, 128:132], Sbv)
                pacc = pse.tile([128, 132], F32, space="PSUM", tag="pacc")
                for cc in range(chb):
                    nc.tensor.matmul(pacc[:], lhsT=ohv[:, cc, :], rhs=SCv[:, cc, :],
                                     start=(cc == 0), stop=(cc == chb - 1))
                # add self loops, normalize, bias, relu, transpose
                nc.vector.tensor_tensor(out=pacc[:], in0=pacc[:],
                                        in1=sSCv[:, b_, :], op=ALU.add)
                dn = sbq.tile([128, 4], F32, tag="dn")
                nc.vector.tensor_scalar_add(dn[:], pacc[:, 128:132], 1e-16)
                nc.vector.reciprocal(dn[:], dn[:])
                gn = sbq.tile([128, 128], F32, tag="gn")
                g4 = gn[:].rearrange("p (h c) -> p h c", h=HEADS)
                p4 = pacc[:, 0:128].rearrange("p (h c) -> p h c", h=HEADS)
                d4 = dn[:].rearrange("p (h w) -> p h w", w=1) \
                    .to_broadcast([128, HEADS, C])
                nc.vector.tensor_tensor(out=g4, in0=p4, in1=d4, op=ALU.mult)
                nc.vector.tensor_tensor(out=gn[:], in0=gn[:], in1=gbr_t[:], op=ALU.add)
                gnb = sbq.tile([128, 128], BF16, tag="gnb")
                nc.vector.tensor_scalar_max(gnb[:], gn[:], 0.0)
                pgT = psk.tile([128, 128], BF16, space="PSUM", tag="ptr")
                nc.tensor.transpose(out=pgT[:], in_=gnb[:], identity=identB[:])
                nc.vector.tensor_copy(gatT[:, b_ * 128:(b_ + 1) * 128], pgT[:])

        # ---------- Phase 3: gx = Wih.T @ gatT (+ warmup mask), bf16 ----
        persist = top.enter_context(tc.tile_pool(name="persist", bufs=1))
        Gxb = persist.tile([128, NTL], BF16)
        H = persist.tile([HID, NTL], BF16)
        nc.gpsimd.memset(H[:], 0.0)
        GSL = [(0, 512), (512, 1024), (1024, NTL)]
        with ExitStack() as ph:
            psg = ph.enter_context(tc.tile_pool(name="psg", bufs=3, space="PSUM"))
            for lo, hi in GSL:
                pgx = psg.tile([128, 512], F32, space="PSUM", tag="pgx")
                nc.tensor.matmul(pgx[:, 0:hi - lo], lhsT=wih_t[:],
                                 rhs=gatT[:, 32 + lo:32 + hi],
                                 start=True, stop=True)
                nc.vector.tensor_copy(Gxb[:, lo:hi], pgx[:, 0:hi - lo])
            nc.vector.tensor_tensor(out=Gxb[:, 0:NW], in0=Gxb[:, 0:NW],
                                    in1=bw_t[:], op=ALU.add)

        # ---------- Phase 4: LSTM fixed point ----------
        # One chained [32, NTL] sequence; warmup absorbs the halo boundary.
        # gx lands in PSUM via a PE identity matmul so the Whh matmul can
        # accumulate onto it within a normal PE accumulation group.
        YS = persist.tile([HID, SEQ], BF16)
        HB = 688                     # column-half boundary for pipelining
        GSLH = [[(0, 512), (512, HB)], [(HB, 1024), (1024, NTL)]]
        with ExitStack() as ph:
            sbl = ph.enter_context(tc.tile_pool(name="sbl", bufs=2))
            psl = ph.enter_context(tc.tile_pool(name="psl", bufs=2, space="PSUM"))
            for it in range(ITERS):
                pG = psl.tile([128, 2048], F32, space="PSUM", tag="pG")
                S_ = sbl.tile([96, NTL], BF16, tag="S")
                Tg = sbl.tile([64, NTL], BF16, tag="Tg")
                Zt = sbl.tile([HID, NTL], BF16, tag="Zt")
                Ct = sbl.tile([HID, NTL], BF16, tag="Ct")
                TCu = sbl.tile([96, NTL], BF16, tag="TCu")
                for hf in range(2):
                    h0 = hf * HB
                    h1 = HB if hf == 0 else NTL
                    for lo, hi in GSLH[hf]:
                        nc.tensor.matmul(pG[:, lo:hi], lhsT=identB[:],
                                         rhs=Gxb[:, lo:hi], start=True,
                                         stop=(it == 0))
                    if it > 0:
                        for lo, hi in GSLH[hf]:
                            nc.tensor.matmul(pG[:, lo:hi], lhsT=whh_t[:],
                                             rhs=H[:, lo:hi], start=False,
                                             stop=True)
                    nc.scalar.activation(S_[:, h0:h1], pG[0:96, h0:h1],
                                         AF.Sigmoid, bias=br_t[0:96, :])
                    nc.scalar.activation(Tg[32:64, h0:h1], pG[96:128, h0:h1],
                                         AF.Tanh, bias=br_t[96:128, :])
                    nc.vector.tensor_tensor(out=Zt[:, h0:h1],
                                            in0=S_[32:64, h0:h1],
                                            in1=Tg[32:64, h0:h1], op=ALU.mult)
                    nc.vector.tensor_tensor_scan(
                        out=Ct[:, h0:h1], data0=S_[0:32, h0:h1],
                        data1=Zt[:, h0:h1],
                        initial=(0.0 if hf == 0 else Ct[:, HB - 1:HB]),
                        op0=ALU.mult, op1=ALU.add)
                    nc.scalar.activation(TCu[64:96, h0:h1], Ct[:, h0:h1],
                                         AF.Tanh)
                    if it < ITERS - 1:
                        # h feeds gates one step later; the last column of
                        # the sequence is never consumed
                        he = h1 + 1 if hf == 0 else h1
                        nc.vector.tensor_tensor(
                            out=H[0:32, h0 + 1:he],
                            in0=S_[64:96, h0:he - 1],
                            in1=TCu[64:96, h0:he - 1], op=ALU.mult)
                    else:
                        y0 = 0 if hf == 0 else HB - NW
                        nc.vector.tensor_tensor(
                            out=YS[:, y0:h1 - NW],
                            in0=S_[64:96, max(h0, NW):h1],
                            in1=TCu[64:96, max(h0, NW):h1], op=ALU.mult)

        # ---------- Phase 5: FC ----------
        with ExitStack() as ph:
            sbf = ph.enter_context(tc.tile_pool(name="sbf", bufs=1))
            psf = ph.enter_context(tc.tile_pool(name="psf", bufs=4, space="PSUM"))
            OF = sbf.tile([1, SEQ], F32)
            for q in range(4):
                pf = psf.tile([1, 320], F32, space="PSUM", tag="pfc")
                nc.tensor.matmul(pf[:], lhsT=wfc_t[:],
                                 rhs=YS[:, q * 320:(q + 1) * 320],
                                 start=True, stop=True)
                nc.vector.tensor_scalar_add(OF[:, q * 320:(q + 1) * 320],
                                            pf[:], bfc_t[:])
            nc.sync.dma_start(out[:], OF[:])

    nc.compile()
    return nc


def run(inputs, trace=False):
    in_maps, chb = _prep_host(inputs)
    if chb not in _CACHE:
        _CACHE[chb] = _build_nc(chb)
    nc = _CACHE[chb]
    res = run_bass_kernel_spmd(nc, in_maps, list(range(NCORES)), trace=trace)
    return res


def kernel(**inputs) -> np.ndarray:
    res = run(inputs)
    full = np.concatenate([np.asarray(res.results[k]["out"][0], np.float32)
                           for k in range(NCORES)])
    return np.ascontiguousarray(full[:N].reshape(N, 1))


# revision 24
# speedup vs baseline: 1.0008x; 1.0008x over previous
"""GAT+LSTM fused kernel for 8 trn2 NeuronCores.

Key structure (per core, fully collective-free):
- The reference output depends only on batch row T-1=11 of its LSTM, so
  only GAT outputs for live nodes [110000, 120000) are needed.
- Live nodes split into 80 buckets of 128 by dst>>7. Core k owns buckets
  [10k-1, 10k+10): its 1280 output nodes PLUS the bucket containing its
  96-step LSTM warmup window (recomputed redundantly, so no cross-core
  exchange is needed anywhere; the host concatenates per-core outputs).
- Edges partitioned by dst bucket, 128-edge chunks. Per chunk:
  indirect-gather x[src] rows (bf16, 128B), PE pair-transpose, h|a_src
  via one bf16 matmul against [W_gat | W_gat@A_src | W_gat@A_dst],
  a_dst via a host-shipped transposed one-hot matmul, segment softmax
  without max subtraction (exp via sigmoid(x)/sigmoid(-x); the Exp ACT
  table is not resident), one-hot scatter matmul accumulating [dst,132]
  in PSUM. Multi-offset indirect DMAs corrupt data at these row sizes
  (verified empirically), hence one 128-offset gather per chunk.
- Self-loops handled densely from the core's own node block (no gather);
  their edge_attr is mean(edge_attr), reduced locally from the
  replicated full edge_attr - no AllReduce.
- LSTM: one chained [32, 1376] sequence (1280 cols + 96 warmup); the
  c-scan chains across the halo boundary, which the warmup absorbs
  (state memory here is only a few dozen steps; f-gates ~ sigmoid(small)).
  Fixed point, ITERS=4 (iteration error ~5e-4, far under the bf16 noise
  floor ~8e-3; tolerance 2e-2). gx lands in PSUM via a PE identity
  matmul so the Whh matmul accumulates onto it in one accumulation
  group (a cross-engine vector preload raced and gave wrong results).
- FC over the 1280 main columns; host concatenates the 8 slices.
"""
import os
import numpy as np
import ml_dtypes

import concourse.bass as bass
import concourse.bacc as bacc
import concourse.tile as tile
from concourse import mybir
from concourse.bass_utils import run_bass_kernel_spmd
from concourse.masks import make_identity
from contextlib import ExitStack

dt = mybir.dt
F32 = dt.float32
BF16 = dt.bfloat16
AF = mybir.ActivationFunctionType
ALU = mybir.AluOpType

T, N, F_IN = 12, 10000, 64
HEADS, C, HID = 4, 32, 32
E, TN = 1_000_000, 120_000
NCORES = 8
D0 = (T - 1) * N
DN = N
NB = 11                      # buckets per core (1 halo + 10 own)
NW = 96                      # LSTM warmup steps
SEQ = 1280                   # sequence cols owned per core
NTL = SEQ + NW               # 1376: one chained sequence incl warmup
ITERS = 4
LEAK = 0.2
XPAD = TN + 64               # x table rows (pad rows are zero)
EAC = (E + 127) // 128       # 7813 cols for the local edge_attr reduce

_CACHE = {}


def _prep_host(inputs):
    x = np.asarray(inputs["x_seq"], np.float32).reshape(TN, F_IN)
    ei = np.asarray(inputs["edge_index"])
    ea = np.asarray(inputs["edge_attr"], np.float32)[:, 0]
    W_gat = np.asarray(inputs["W_gat"], np.float32)
    att_src = np.asarray(inputs["att_src"], np.float32)
    att_dst = np.asarray(inputs["att_dst"], np.float32)
    att_edge = np.asarray(inputs["att_edge"], np.float32)
    W_edge = np.asarray(inputs["W_edge"], np.float32)
    gat_bias = np.asarray(inputs["gat_bias"], np.float32)
    W_ih = np.asarray(inputs["W_ih"], np.float32)
    W_hh = np.asarray(inputs["W_hh"], np.float32)
    b = np.asarray(inputs["b_ih"], np.float32) + np.asarray(inputs["b_hh"], np.float32)
    W_fc = np.asarray(inputs["W_fc"], np.float32)
    b_fc = np.asarray(inputs["b_fc"], np.float32)

    # W_all: [64, 136] = [W_gat | W_gat@A_src | W_gat@A_dst]
    A_src = np.zeros((HEADS * C, HEADS), np.float32)
    A_dst = np.zeros((HEADS * C, HEADS), np.float32)
    for h in range(HEADS):
        A_src[h * C:(h + 1) * C, h] = att_src[h]
        A_dst[h * C:(h + 1) * C, h] = att_dst[h]
    W_all = np.concatenate([W_gat, W_gat @ A_src, W_gat @ A_dst], axis=1)
    kap = np.array([np.dot(W_edge[0, h * C:(h + 1) * C], att_edge[h])
                    for h in range(HEADS)], np.float32)
    kap_rep = np.broadcast_to(kap, (128, HEADS)).copy()
    gb_row = np.broadcast_to(gat_bias, (128, HEADS * C)).copy()
    iota128 = np.broadcast_to(np.arange(128, dtype=np.float32), (128, 128)).copy()
    # gate row order [f, i, o, g] (torch order is i,f,g,o)
    perm = np.concatenate([np.arange(32, 64), np.arange(0, 32),
                           np.arange(96, 128), np.arange(64, 96)])
    WihT = np.ascontiguousarray(W_ih[perm].T)
    WhhT = np.ascontiguousarray(W_hh[perm].T)
    br = np.ascontiguousarray(b[perm].reshape(128, 1))

    xbf = np.zeros((XPAD, F_IN), ml_dtypes.bfloat16)
    xbf[:TN] = x
    eaF = np.zeros((128, EAC), np.float32)
    j = np.arange(E)
    eaF[j % 128, j // 128] = ea

    src = ei[0].astype(np.int64)
    dst = ei[1].astype(np.int64)
    live = (dst >= D0) & (dst < D0 + DN)
    sl = src[live]
    dl = dst[live] - D0
    bkt = dl >> 7
    cnt = np.bincount(bkt, minlength=80)
    chb = int(max(1, -(-int(cnt.max()) // 128)))   # chunks per bucket
    nch = NB * chb

    by_bucket = {}
    order = np.argsort(bkt, kind="stable")
    pos = 0
    for gb in range(80):
        by_bucket[gb] = order[pos:pos + cnt[gb]]
        pos += cnt[gb]

    eal = ea[live]
    in_maps = []
    for k in range(NCORES):
        xI = np.full((128, nch), TN, np.int32)
        eaC = np.zeros((128, nch), np.float32)
        dstF = np.full((128, nch), -1.0, np.float32)
        ohT = np.zeros((128, nch * 128), ml_dtypes.bfloat16)
        for b_ in range(NB):
            gb = 10 * k - 1 + b_
            if not (0 <= gb < 80):
                continue
            sel = by_bucket[gb]
            ne = len(sel)
            assert ne <= chb * 128
            ii = np.arange(ne)
            cc = b_ * chb + ii // 128
            pp = ii % 128
            xI[pp, cc] = sl[sel].astype(np.int32)
            eaC[pp, cc] = eal[sel]
            dpos = (dl[sel] - 128 * gb).astype(np.int32)
            dstF[pp, cc] = dpos.astype(np.float32)
            ohT[dpos, cc * 128 + pp] = 1.0
        # own node features, transposed: bucket-local nodes [128*(10k-1), +1408)
        xTD = np.zeros((F_IN, NB * 128), ml_dtypes.bfloat16)
        lo = 128 * (10 * k - 1)
        for j2 in range(NB * 128):
            gn = lo + j2
            if 0 <= gn < DN:
                xTD[:, j2] = x[D0 + gn]
        Bwarm = np.zeros((128, NW), np.float32)
        if k == 0:
            Bwarm[32:96, :] = -30.0    # i and o gate rows of the junk warmup
        in_maps.append({
            "xnd": xbf, "eaF": eaF,
            "xI": xI, "ohT": ohT, "eaC": eaC, "dstF": dstF,
            "xTD": np.ascontiguousarray(xTD),
            "Bwarm": Bwarm.astype(ml_dtypes.bfloat16),
            "Wall": W_all.astype(ml_dtypes.bfloat16),
            "kap": kap_rep, "gbrow": gb_row, "iota": iota128,
            "Wih": WihT.astype(ml_dtypes.bfloat16),
            "Whh": WhhT.astype(ml_dtypes.bfloat16),
            "br": br,
            "Wfc": np.ascontiguousarray(W_fc.reshape(HID, 1)).astype(ml_dtypes.bfloat16),
            "bfc": np.ascontiguousarray(b_fc.reshape(1, 1)),
        })
    return in_maps, chb


def _build_nc(chb):
    nch = NB * chb
    nc = bacc.Bacc("TRN2", target_bir_lowering=False, debug=False,
                   num_devices=NCORES)
    g = lambda n, s, d=F32: nc.dram_tensor(n, s, d, kind="ExternalInput").ap()
    xnd = g("xnd", [XPAD, F_IN], BF16)
    eaF = g("eaF", [128, EAC])
    xI = g("xI", [128, nch], dt.int32)
    ohT = g("ohT", [128, nch * 128], BF16)
    eaC = g("eaC", [128, nch])
    dstF = g("dstF", [128, nch])
    xTD = g("xTD", [F_IN, NB * 128], BF16)
    Bwarm = g("Bwarm", [128, NW], BF16)
    Wall = g("Wall", [F_IN, 136], BF16)
    kap = g("kap", [128, HEADS])
    gbrow = g("gbrow", [128, 128])
    iota = g("iota", [128, 128])
    Wih = g("Wih", [128, 128], BF16)
    Whh = g("Whh", [HID, 128], BF16)
    br = g("br", [128, 1])
    Wfc = g("Wfc", [HID, 1], BF16)
    bfc = g("bfc", [1, 1])
    out = nc.dram_tensor("out", [1, SEQ], F32, kind="ExternalOutput").ap()

    with tile.TileContext(nc) as tc, ExitStack() as top:
        const = top.enter_context(tc.tile_pool(name="const", bufs=1))
        identB = const.tile([128, 128], BF16)
        make_identity(nc, identB[:])
        wall_t = const.tile([F_IN, 136], BF16); nc.sync.dma_start(wall_t[:], Wall[:])
        kap_t = const.tile([128, HEADS], F32); nc.sync.dma_start(kap_t[:], kap[:])
        gbr_t = const.tile([128, 128], F32); nc.sync.dma_start(gbr_t[:], gbrow[:])
        iota_t = const.tile([128, 128], F32); nc.sync.dma_start(iota_t[:], iota[:])
        wih_t = const.tile([128, 128], BF16); nc.sync.dma_start(wih_t[:], Wih[:])
        whh_t = const.tile([HID, 128], BF16); nc.sync.dma_start(whh_t[:], Whh[:])
        br_t = const.tile([128, 1], F32); nc.sync.dma_start(br_t[:], br[:])
        wfc_t = const.tile([HID, 1], BF16); nc.sync.dma_start(wfc_t[:], Wfc[:])
        bfc_t = const.tile([1, 1], F32); nc.sync.dma_start(bfc_t[:], bfc[:])
        bw_t = const.tile([128, NW], BF16); nc.sync.dma_start(bw_t[:], Bwarm[:])
        xi_t = const.tile([128, nch], dt.int32); nc.sync.dma_start(xi_t[:], xI[:])
        ohT_t = const.tile([128, nch * 128], BF16); nc.sync.dma_start(ohT_t[:], ohT[:])
        eac_t = const.tile([128, nch], F32); nc.sync.dma_start(eac_t[:], eaC[:])
        dsf_t = const.tile([128, nch], F32); nc.sync.dma_start(dsf_t[:], dstF[:])
        xtd_t = const.tile([F_IN, NB * 128], BF16); nc.sync.dma_start(xtd_t[:], xTD[:])
        meanr = const.tile([128, 1], F32)
        gatT = const.tile([128, NB * 128], BF16)     # [feat, bucket-local node]

        # ---------- Phase 0: mean(edge_attr), local full reduce ----------
        with ExitStack() as ph:
            sbm = ph.enter_context(tc.tile_pool(name="sbm", bufs=1))
            psm = ph.enter_context(tc.tile_pool(name="psm", bufs=1, space="PSUM"))
            eaf_t = sbm.tile([128, EAC], F32)
            nc.sync.dma_start(eaf_t[:], eaF[:])
            eap = sbm.tile([128, 1], F32)
            nc.vector.tensor_reduce(eap[:], eaf_t[:], mybir.AxisListType.X, ALU.add)
            onc = sbm.tile([128, 1], F32)
            nc.gpsimd.memset(onc[:], 1.0)
            ps1 = psm.tile([1, 1], F32, space="PSUM", tag="ps1")
            nc.tensor.matmul(ps1[:], lhsT=eap[:], rhs=onc[:], start=True, stop=True)
            eas = sbm.tile([1, 1], F32)
            nc.scalar.mul(eas[:], ps1[:], 1.0 / E)
            onr = sbm.tile([1, 128], F32)
            nc.gpsimd.memset(onr[:], 1.0)
            ps2 = psm.tile([128, 1], F32, space="PSUM", tag="ps2")
            nc.tensor.matmul(ps2[:], lhsT=onr[:], rhs=eas[:], start=True, stop=True)
            nc.vector.tensor_copy(meanr[:], ps2[:])

        # ---------- Phase 1: self tables (h|a_src|a_dst for own nodes) ----
        sfp = top.enter_context(tc.tile_pool(name="sfp", bufs=1))
        SF = sfp.tile([128, NB * 136], F32)
        SFv = SF[:].rearrange("p (j w) -> p j w", w=136)
        adB = sfp.tile([128, NB * 4], BF16)
        adBv = adB[:].rearrange("p (j w) -> p j w", w=4)
        selfSC = sfp.tile([128, NB * 132], BF16)
        sSCv = selfSC[:].rearrange("p (j w) -> p j w", w=132)
        with ExitStack() as ph:
            sbs = ph.enter_context(tc.tile_pool(name="sbs", bufs=1))
            pss = ph.enter_context(tc.tile_pool(name="pss", bufs=4, space="PSUM"))
            for b_ in range(NB):
                pf = pss.tile([128, 136], F32, space="PSUM", tag="pf")
                nc.tensor.matmul(pf[:], lhsT=xtd_t[:, b_ * 128:(b_ + 1) * 128],
                                 rhs=wall_t[:], start=True, stop=True)
                nc.vector.tensor_copy(SFv[:, b_, :], pf[:])
            nc.vector.tensor_copy(adBv, SFv[:, :, 132:136])
            QS = sbs.tile([128, NB * 4], F32)
            QSv = QS[:].rearrange("p (j w) -> p j w", w=4)
            nc.vector.tensor_tensor(out=QSv, in0=SFv[:, :, 128:132],
                                    in1=SFv[:, :, 132:136], op=ALU.add)
            kapb = kap_t[:].rearrange("p (o w) -> p o w", o=1) \
                .to_broadcast([128, NB, 4])
            nc.vector.scalar_tensor_tensor(out=QSv, in0=kapb, scalar=meanr[:],
                                           op0=ALU.mult, op1=ALU.add, in1=QSv)
            T2 = sbs.tile([128, NB * 4], F32)
            nc.vector.tensor_scalar_mul(T2[:], QS[:], LEAK)
            nc.vector.tensor_tensor(out=QS[:], in0=QS[:], in1=T2[:], op=ALU.max)
            SG1 = sbs.tile([128, NB * 4], F32)
            nc.scalar.activation(SG1[:], QS[:], AF.Sigmoid)
            nc.scalar.activation(T2[:], QS[:], AF.Sigmoid, scale=-1.0)
            nc.vector.reciprocal(T2[:], T2[:])
            SSf = sbs.tile([128, NB * 4], F32)
            nc.vector.tensor_tensor(out=SSf[:], in0=SG1[:], in1=T2[:], op=ALU.mult)
            ssf4 = SSf[:].rearrange("p (j h w) -> p j h w", h=HEADS, w=1) \
                .to_broadcast([128, NB, HEADS, C])
            sf4 = SFv[:, :, 0:128].rearrange("p j (h c) -> p j h c", h=HEADS)
            o4 = sSCv[:, :, 0:128].rearrange("p j (h c) -> p j h c", h=HEADS)
            nc.vector.tensor_tensor(out=o4, in0=sf4, in1=ssf4, op=ALU.mult)
            nc.vector.tensor_copy(sSCv[:, :, 128:132],
                                  SSf[:].rearrange("p (j w) -> p j w", w=4))

        # ---------- Phase 2: edge phase ----------
        NXT = (chb + 1) // 2          # transpose pair count per bucket
        with ExitStack() as ph:
            sbe = ph.enter_context(tc.tile_pool(name="sbe", bufs=2))
            sbq = ph.enter_context(tc.tile_pool(name="sbq", bufs=2))
            pse = ph.enter_context(tc.tile_pool(name="pse", bufs=2, space="PSUM"))
            psh = ph.enter_context(tc.tile_pool(name="psh", bufs=2, space="PSUM"))
            psa = ph.enter_context(tc.tile_pool(name="psa", bufs=2, space="PSUM"))
            psk = ph.enter_context(tc.tile_pool(name="psk", bufs=2, space="PSUM"))
            for b_ in range(NB):
                c0 = b_ * chb
                XGBt = sbe.tile([128, chb * 64], BF16, tag="XGB")
                XGBv = XGBt[:].rearrange("p (e w) -> p e w", w=64)
                for cc in range(chb):
                    nc.gpsimd.indirect_dma_start(
                        out=XGBv[:, cc, :], out_offset=None, in_=xnd[:],
                        in_offset=bass.IndirectOffsetOnAxis(
                            ap=xi_t[:, c0 + cc:c0 + cc + 1], axis=0))
                XGB = XGBt[:]
                XT = sbe.tile([F_IN, chb * 128], BF16, tag="XT")
                for pr in range(NXT):
                    w = min(128, chb * 64 - pr * 128)
                    pxT = psk.tile([128, 128], BF16, space="PSUM", tag="ptr")
                    nc.tensor.transpose(out=pxT[0:w, :],
                                        in_=XGB[:, pr * 128:pr * 128 + w],
                                        identity=identB[:])
                    nc.vector.tensor_copy(XT[:, (2 * pr) * 128:(2 * pr + 1) * 128],
                                          pxT[0:64, :])
                    if w > 64:
                        nc.vector.tensor_copy(
                            XT[:, (2 * pr + 1) * 128:(2 * pr + 2) * 128],
                            pxT[64:128, :])
                # h|a_src per edge; batches of 3 chunks share one PSUM tile
                hEs = sbe.tile([128, chb * 132], BF16, tag="hEs")
                hEv = hEs[:].rearrange("p (e w) -> p e w", w=132)
                Qb = sbq.tile([128, chb * 4], F32, tag="Qb")
                Qv = Qb[:].rearrange("p (e w) -> p e w", w=4)
                nb3 = (chb + 2) // 3
                for b3 in range(nb3):
                    n3 = min(3, chb - b3 * 3)
                    phE = psh.tile([128, 3 * 132], F32, space="PSUM", tag="phE")
                    for j3 in range(n3):
                        cc = b3 * 3 + j3
                        lhs = XT[:, cc * 128:(cc + 1) * 128]
                        nc.tensor.matmul(phE[:, j3 * 132:(j3 + 1) * 132],
                                         lhsT=lhs, rhs=wall_t[:, 0:132],
                                         start=True, stop=True)
                    phEv = phE[:].rearrange("p (e w) -> p e w", w=132)
                    nc.vector.tensor_copy(hEv[:, b3 * 3:b3 * 3 + n3, :],
                                          phEv[:, 0:n3, :])
                    nc.vector.tensor_copy(Qv[:, b3 * 3:b3 * 3 + n3, :],
                                          phEv[:, 0:n3, 128:132])
                # one-hots for the whole bucket in one op
                ohs = sbe.tile([128, chb * 128], BF16, tag="ohs")
                ohv = ohs[:].rearrange("p (e w) -> p e w", w=128)
                dfb = dsf_t[:, c0:c0 + chb].rearrange("p (e w) -> p e w", w=1) \
                    .to_broadcast([128, chb, 128])
                iob = iota_t[:].rearrange("p (o w) -> p o w", o=1) \
                    .to_broadcast([128, chb, 128])
                nc.vector.tensor_tensor(out=ohv, in0=dfb, in1=iob, op=ALU.is_equal)
                # a_dst per edge: one-hot-transpose (host-shipped) matmul
                padc = psa.tile([128, chb * 4], F32, space="PSUM", tag="padc")
                for cc in range(chb):
                    nc.tensor.matmul(
                        padc[:, cc * 4:(cc + 1) * 4],
                        lhsT=ohT_t[:, (c0 + cc) * 128:(c0 + cc + 1) * 128],
                        rhs=adBv[:, b_, :], start=True, stop=True)
                # q = a_src + a_dst + ea*kap ; s = exp(leaky_relu(q))
                kmb = sbq.tile([128, chb * 4], F32, tag="kmb")
                kmv = kmb[:].rearrange("p (e w) -> p e w", w=4)
                eab = eac_t[:, c0:c0 + chb].rearrange("p (e w) -> p e w", w=1) \
                    .to_broadcast([128, chb, 4])
                kab = kap_t[:].rearrange("p (o w) -> p o w", o=1) \
                    .to_broadcast([128, chb, 4])
                nc.vector.tensor_tensor(out=kmv, in0=eab, in1=kab, op=ALU.mult)
                nc.vector.tensor_tensor(out=Qb[:], in0=Qb[:], in1=kmb[:], op=ALU.add)
                nc.vector.tensor_tensor(out=Qb[:], in0=Qb[:], in1=padc[:],
                                        op=ALU.add)
                nc.vector.tensor_scalar_mul(kmb[:], Qb[:], LEAK)
                nc.vector.tensor_tensor(out=Qb[:], in0=Qb[:], in1=kmb[:], op=ALU.max)
                SG = sbq.tile([128, chb * 4], F32, tag="SG")
                nc.scalar.activation(SG[:], Qb[:], AF.Sigmoid)
                nc.scalar.activation(kmb[:], Qb[:], AF.Sigmoid, scale=-1.0)
                nc.vector.reciprocal(kmb[:], kmb[:])
                Sbf = sbq.tile([128, chb * 4], BF16, tag="Sbf")
                nc.vector.tensor_tensor(out=Sbf[:], in0=SG[:], in1=kmb[:], op=ALU.mult)
                Sbv = Sbf[:].rearrange("p (e w) -> p e w", w=4)
                # messages and scatter
                SCb = sbe.tile([128, chb * 132], BF16, tag="SCb")
                SCv = SCb[:].rearrange("p (e w) -> p e w", w=132)
                sb4 = Sbf[:].rearrange("p (e h w) -> p e h w", h=HEADS, w=1) \
                    .to_broadcast([128, chb, HEADS, C])
                he4 = hEv[:, :, 0:128].rearrange("p e (h c) -> p e h c", h=HEADS)
                sc4 = SCv[:, :, 0:128].rearrange("p e (h c) -> p e h c", h=HEADS)
                nc.vector.tensor_tensor(out=sc4, in0=he4, in1=sb4, op=ALU.mult)
                nc.vector.tensor_copy(SCv[:, :, 128:132], Sbv)
                pacc = pse.tile([128, 132], F32, space="PSUM", tag="pacc")
                for cc in range(chb):
                    nc.tensor.matmul(pacc[:], lhsT=ohv[:, cc, :], rhs=SCv[:, cc, :],
                                     start=(cc == 0), stop=(cc == chb - 1))
                # add self loops, normalize, bias, relu, transpose
                nc.vector.tensor_tensor(out=pacc[:], in0=pacc[:],
                                        in1=sSCv[:, b_, :], op=ALU.add)
                dn = sbq.tile([128, 4], F32, tag="dn")
                nc.vector.tensor_scalar_add(dn[:], pacc[:, 128:132], 1e-16)
                nc.vector.reciprocal(dn[:], dn[:])
                gn = sbq.tile([128, 128], F32, tag="gn")
                g4 = gn[:].rearrange("p (h c) -> p h c", h=HEADS)
                p4 = pacc[:, 0:128].rearrange("p (h c) -> p h c", h=HEADS)
                d4 = dn[:].rearrange("p (h w) -> p h w", w=1) \
                    .to_broadcast([128, HEADS, C])
                nc.vector.tensor_tensor(out=g4, in0=p4, in1=d4, op=ALU.mult)
                nc.vector.tensor_tensor(out=gn[:], in0=gn[:], in1=gbr_t[:], op=ALU.add)
                gnb = sbq.tile([128, 128], BF16, tag="gnb")
                nc.vector.tensor_scalar_max(gnb[:], gn[:], 0.0)
                pgT = psk.tile([128, 128], BF16, space="PSUM", tag="ptr")
                nc.tensor.transpose(out=pgT[:], in_=gnb[:], identity=identB[:])
                nc.vector.tensor_copy(gatT[:, b_ * 128:(b_ + 1) * 128], pgT[:])

        # ---------- Phase 3: gx = Wih.T @ gatT (+ warmup mask), bf16 ----
        persist = top.enter_context(tc.tile_pool(name="persist", bufs=1))
        Gxb = persist.tile([128, NTL], BF16)
        H = persist.tile([HID, NTL], BF16)
        nc.gpsimd.memset(H[:], 0.0)
        GSL = [(0, 512), (512, 1024), (1024, NTL)]
        with ExitStack() as ph:
            psg = ph.enter_context(tc.tile_pool(name="psg", bufs=3, space="PSUM"))
            for lo, hi in GSL:
                pgx = psg.tile([128, 512], F32, space="PSUM", tag="pgx")
                nc.tensor.matmul(pgx[:, 0:hi - lo], lhsT=wih_t[:],
                                 rhs=gatT[:, 32 + lo:32 + hi],
                                 start=True, stop=True)
                nc.vector.tensor_copy(Gxb[:, lo:hi], pgx[:, 0:hi - lo])
            nc.vector.tensor_tensor(out=Gxb[:, 0:NW], in0=Gxb[:, 0:NW],
                                    in1=bw_t[:], op=ALU.add)

        # ---------- Phase 4: LSTM fixed point ----------
        # One chained [32, NTL] sequence; warmup absorbs the halo boundary.
        # gx lands in PSUM via a PE identity matmul so the Whh matmul can
        # accumulate onto it within a normal PE accumulation group.
        YS = persist.tile([HID, SEQ], BF16)
        HB = 688                     # column-half boundary for pipelining
        GSLH = [[(0, 512), (512, HB)], [(HB, 1024), (1024, NTL)]]
        with ExitStack() as ph:
            sbl = ph.enter_context(tc.tile_pool(name="sbl", bufs=2))
            psl = ph.enter_context(tc.tile_pool(name="psl", bufs=2, space="PSUM"))
            for it in range(ITERS):
                pG = psl.tile([128, 2048], F32, space="PSUM", tag="pG")
                S_ = sbl.tile([96, NTL], BF16, tag="S")
                Tg = sbl.tile([64, NTL], BF16, tag="Tg")
                Zt = sbl.tile([HID, NTL], BF16, tag="Zt")
                Ct = sbl.tile([HID, NTL], BF16, tag="Ct")
                TCu = sbl.tile([96, NTL], BF16, tag="TCu")
                for hf in range(2):
                    h0 = hf * HB
                    h1 = HB if hf == 0 else NTL
                    for lo, hi in GSLH[hf]:
                        nc.tensor.matmul(pG[:, lo:hi], lhsT=identB[:],
                                         rhs=Gxb[:, lo:hi], start=True,
                                         stop=(it == 0))
                    if it > 0:
                        for lo, hi in GSLH[hf]:
                            nc.tensor.matmul(pG[:, lo:hi], lhsT=whh_t[:],
                                             rhs=H[:, lo:hi], start=False,
                                             stop=True)
                    nc.scalar.activation(S_[:, h0:h1], pG[0:96, h0:h1],
                                         AF.Sigmoid, bias=br_t[0:96, :])
                    nc.scalar.activation(Tg[32:64, h0:h1], pG[96:128, h0:h1],
                                         AF.Tanh, bias=br_t[96:128, :])
                    nc.vector.tensor_tensor(out=Zt[:, h0:h1],
                                            in0=S_[32:64, h0:h1],
                                            in1=Tg[32:64, h0:h1], op=ALU.mult)
                    nc.vector.tensor_tensor_scan(
                        out=Ct[:, h0:h1], data0=S_[0:32, h0:h1],
                        data1=Zt[:, h0:h1],
                        initial=(0.0 if hf == 0 else Ct[:, HB - 1:HB]),
                        op0=ALU.mult, op1=ALU.add)
                    nc.scalar.activation(TCu[64:96, h0:h1], Ct[:, h0:h1],
                                         AF.Tanh)
                    if it < ITERS - 1:
                        # h feeds gates one step later; the last column of
                        # the sequence is never consumed
                        he = h1 + 1 if hf == 0 else h1
                        nc.gpsimd.tensor_tensor(
                            out=H[0:32, h0 + 1:he],
                            in0=S_[64:96, h0:he - 1],
                            in1=TCu[64:96, h0:he - 1], op=ALU.mult)
                    else:
                        y0 = 0 if hf == 0 else HB - NW
                        nc.vector.tensor_tensor(
                            out=YS[:, y0:h1 - NW],
                            in0=S_[64:96, max(h0, NW):h1],
                            in1=TCu[64:96, max(h0, NW):h1], op=ALU.mult)

        # ---------- Phase 5: FC ----------
        with ExitStack() as ph:
            sbf = ph.enter_context(tc.tile_pool(name="sbf", bufs=1))
            psf = ph.enter_context(tc.tile_pool(name="psf", bufs=4, space="PSUM"))
            OF = sbf.tile([1, SEQ], F32)
            for q in range(4):
                pf = psf.tile([1, 320], F32, space="PSUM", tag="pfc")
                nc.tensor.matmul(pf[:], lhsT=wfc_t[:],
                                 rhs=YS[:, q * 320:(q + 1) * 320],
                                 start=True, stop=True)
                nc.vector.tensor_scalar_add(OF[:, q * 320:(q + 1) * 320],
                                            pf[:], bfc_t[:])
            nc.sync.dma_start(out[:], OF[:])

    nc.compile()
    return nc


def run(inputs, trace=False):
    in_maps, chb = _prep_host(inputs)
    if chb not in _CACHE:
        _CACHE[chb] = _build_nc(chb)
    nc = _CACHE[chb]
    res = run_bass_kernel_spmd(nc, in_maps, list(range(NCORES)), trace=trace)
    return res


def kernel(**inputs) -> np.ndarray:
    res = run(inputs)
    full = np.concatenate([np.asarray(res.results[k]["out"][0], np.float32)
                           for k in range(NCORES)])
    return np.ascontiguousarray(full[:N].reshape(N, 1))


# revision 25
# speedup vs baseline: 1.0039x; 1.0030x over previous
"""GAT+LSTM fused kernel for 8 trn2 NeuronCores.

Key structure (per core, fully collective-free):
- The reference output depends only on batch row T-1=11 of its LSTM, so
  only GAT outputs for live nodes [110000, 120000) are needed.
- Live nodes split into 80 buckets of 128 by dst>>7. Core k owns buckets
  [10k-1, 10k+10): its 1280 output nodes PLUS the bucket containing its
  96-step LSTM warmup window (recomputed redundantly, so no cross-core
  exchange is needed anywhere; the host concatenates per-core outputs).
- Edges partitioned by dst bucket, 128-edge chunks. Per chunk:
  indirect-gather x[src] rows (bf16, 128B), PE pair-transpose, h|a_src
  via one bf16 matmul against [W_gat | W_gat@A_src | W_gat@A_dst],
  a_dst via a host-shipped transposed one-hot matmul, segment softmax
  without max subtraction (exp via sigmoid(x)/sigmoid(-x); the Exp ACT
  table is not resident), one-hot scatter matmul accumulating [dst,132]
  in PSUM. Multi-offset indirect DMAs corrupt data at these row sizes
  (verified empirically), hence one 128-offset gather per chunk.
- Self-loops handled densely from the core's own node block (no gather);
  their edge_attr is mean(edge_attr), reduced locally from the
  replicated full edge_attr - no AllReduce.
- LSTM: one chained [32, 1376] sequence (1280 cols + 96 warmup); the
  c-scan chains across the halo boundary, which the warmup absorbs
  (state memory here is only a few dozen steps; f-gates ~ sigmoid(small)).
  Fixed point, ITERS=4 (iteration error ~5e-4, far under the bf16 noise
  floor ~8e-3; tolerance 2e-2). gx lands in PSUM via a PE identity
  matmul so the Whh matmul accumulates onto it in one accumulation
  group (a cross-engine vector preload raced and gave wrong results).
- FC over the 1280 main columns; host concatenates the 8 slices.
"""
import os
import numpy as np
import ml_dtypes

import concourse.bass as bass
import concourse.bacc as bacc
import concourse.tile as tile
from concourse import mybir
from concourse.bass_utils import run_bass_kernel_spmd
from concourse.masks import make_identity
from contextlib import ExitStack

dt = mybir.dt
F32 = dt.float32
BF16 = dt.bfloat16
FP8 = dt.float8e4
AF = mybir.ActivationFunctionType
ALU = mybir.AluOpType

T, N, F_IN = 12, 10000, 64
HEADS, C, HID = 4, 32, 32
E, TN = 1_000_000, 120_000
NCORES = 8
D0 = (T - 1) * N
DN = N
NB = 11                      # buckets per core (1 halo + 10 own)
NW = 96                      # LSTM warmup steps
SEQ = 1280                   # sequence cols owned per core
NTL = SEQ + NW               # 1376: one chained sequence incl warmup
ITERS = 4
LEAK = 0.2
XPAD = TN + 64               # x table rows (pad rows are zero)
EAC = (E + 127) // 128       # 7813 cols for the local edge_attr reduce

_CACHE = {}


def _prep_host(inputs):
    x = np.asarray(inputs["x_seq"], np.float32).reshape(TN, F_IN)
    ei = np.asarray(inputs["edge_index"])
    ea = np.asarray(inputs["edge_attr"], np.float32)[:, 0]
    W_gat = np.asarray(inputs["W_gat"], np.float32)
    att_src = np.asarray(inputs["att_src"], np.float32)
    att_dst = np.asarray(inputs["att_dst"], np.float32)
    att_edge = np.asarray(inputs["att_edge"], np.float32)
    W_edge = np.asarray(inputs["W_edge"], np.float32)
    gat_bias = np.asarray(inputs["gat_bias"], np.float32)
    W_ih = np.asarray(inputs["W_ih"], np.float32)
    W_hh = np.asarray(inputs["W_hh"], np.float32)
    b = np.asarray(inputs["b_ih"], np.float32) + np.asarray(inputs["b_hh"], np.float32)
    W_fc = np.asarray(inputs["W_fc"], np.float32)
    b_fc = np.asarray(inputs["b_fc"], np.float32)

    # W_all: [64, 136] = [W_gat | W_gat@A_src | W_gat@A_dst]
    A_src = np.zeros((HEADS * C, HEADS), np.float32)
    A_dst = np.zeros((HEADS * C, HEADS), np.float32)
    for h in range(HEADS):
        A_src[h * C:(h + 1) * C, h] = att_src[h]
        A_dst[h * C:(h + 1) * C, h] = att_dst[h]
    W_all = np.concatenate([W_gat, W_gat @ A_src, W_gat @ A_dst], axis=1)
    kap = np.array([np.dot(W_edge[0, h * C:(h + 1) * C], att_edge[h])
                    for h in range(HEADS)], np.float32)
    kap_rep = np.broadcast_to(kap, (128, HEADS)).copy()
    gb_row = np.broadcast_to(gat_bias, (128, HEADS * C)).copy()
    iota128 = np.broadcast_to(np.arange(128, dtype=np.float32), (128, 128)).copy()
    # gate row order [f, i, o, g] (torch order is i,f,g,o)
    perm = np.concatenate([np.arange(32, 64), np.arange(0, 32),
                           np.arange(96, 128), np.arange(64, 96)])
    WihT = np.ascontiguousarray(W_ih[perm].T)
    WhhT = np.ascontiguousarray(W_hh[perm].T)
    br = np.ascontiguousarray(b[perm].reshape(128, 1))

    xbf = np.zeros((XPAD, F_IN), ml_dtypes.bfloat16)
    xbf[:TN] = x
    eaF = np.zeros((128, EAC), np.float32)
    j = np.arange(E)
    eaF[j % 128, j // 128] = ea

    src = ei[0].astype(np.int64)
    dst = ei[1].astype(np.int64)
    live = (dst >= D0) & (dst < D0 + DN)
    sl = src[live]
    dl = dst[live] - D0
    bkt = dl >> 7
    cnt = np.bincount(bkt, minlength=80)
    chb = int(max(1, -(-int(cnt.max()) // 128)))   # chunks per bucket
    nch = NB * chb

    by_bucket = {}
    order = np.argsort(bkt, kind="stable")
    pos = 0
    for gb in range(80):
        by_bucket[gb] = order[pos:pos + cnt[gb]]
        pos += cnt[gb]

    eal = ea[live]
    in_maps = []
    for k in range(NCORES):
        xI = np.full((128, nch), TN, np.int32)
        eaC = np.zeros((128, nch), np.float32)
        dstF = np.full((128, nch), -1.0, np.float32)
        ohT = np.zeros((128, nch * 128), ml_dtypes.float8_e4m3)
        for b_ in range(NB):
            gb = 10 * k - 1 + b_
            if not (0 <= gb < 80):
                continue
            sel = by_bucket[gb]
            ne = len(sel)
            assert ne <= chb * 128
            ii = np.arange(ne)
            cc = b_ * chb + ii // 128
            pp = ii % 128
            xI[pp, cc] = sl[sel].astype(np.int32)
            eaC[pp, cc] = eal[sel]
            dpos = (dl[sel] - 128 * gb).astype(np.int32)
            dstF[pp, cc] = dpos.astype(np.float32)
            ohT[dpos, cc * 128 + pp] = 1.0
        # own node features, transposed: bucket-local nodes [128*(10k-1), +1408)
        xTD = np.zeros((F_IN, NB * 128), ml_dtypes.bfloat16)
        lo = 128 * (10 * k - 1)
        for j2 in range(NB * 128):
            gn = lo + j2
            if 0 <= gn < DN:
                xTD[:, j2] = x[D0 + gn]
        Bwarm = np.zeros((128, NW), np.float32)
        if k == 0:
            Bwarm[32:96, :] = -30.0    # i and o gate rows of the junk warmup
        in_maps.append({
            "xnd": xbf, "eaF": eaF,
            "xI": xI, "ohT": ohT, "eaC": eaC, "dstF": dstF,
            "xTD": np.ascontiguousarray(xTD),
            "Bwarm": Bwarm.astype(ml_dtypes.bfloat16),
            "Wall": W_all.astype(ml_dtypes.bfloat16),
            "kap": kap_rep, "gbrow": gb_row, "iota": iota128,
            "Wih": WihT.astype(ml_dtypes.bfloat16),
            "Whh": WhhT.astype(ml_dtypes.bfloat16),
            "br": br,
            "Wfc": np.ascontiguousarray(W_fc.reshape(HID, 1)).astype(ml_dtypes.bfloat16),
            "bfc": np.ascontiguousarray(b_fc.reshape(1, 1)),
        })
    return in_maps, chb


def _build_nc(chb):
    nch = NB * chb
    nc = bacc.Bacc("TRN2", target_bir_lowering=False, debug=False,
                   num_devices=NCORES)
    g = lambda n, s, d=F32: nc.dram_tensor(n, s, d, kind="ExternalInput").ap()
    xnd = g("xnd", [XPAD, F_IN], BF16)
    eaF = g("eaF", [128, EAC])
    xI = g("xI", [128, nch], dt.int32)
    ohT = g("ohT", [128, nch * 128], FP8)
    eaC = g("eaC", [128, nch])
    dstF = g("dstF", [128, nch])
    xTD = g("xTD", [F_IN, NB * 128], BF16)
    Bwarm = g("Bwarm", [128, NW], BF16)
    Wall = g("Wall", [F_IN, 136], BF16)
    kap = g("kap", [128, HEADS])
    gbrow = g("gbrow", [128, 128])
    iota = g("iota", [128, 128])
    Wih = g("Wih", [128, 128], BF16)
    Whh = g("Whh", [HID, 128], BF16)
    br = g("br", [128, 1])
    Wfc = g("Wfc", [HID, 1], BF16)
    bfc = g("bfc", [1, 1])
    out = nc.dram_tensor("out", [1, SEQ], F32, kind="ExternalOutput").ap()

    with tile.TileContext(nc) as tc, ExitStack() as top:
        const = top.enter_context(tc.tile_pool(name="const", bufs=1))
        identB = const.tile([128, 128], BF16)
        make_identity(nc, identB[:])
        wall_t = const.tile([F_IN, 136], BF16); nc.sync.dma_start(wall_t[:], Wall[:])
        kap_t = const.tile([128, HEADS], F32); nc.sync.dma_start(kap_t[:], kap[:])
        gbr_t = const.tile([128, 128], F32); nc.sync.dma_start(gbr_t[:], gbrow[:])
        iota_t = const.tile([128, 128], F32); nc.sync.dma_start(iota_t[:], iota[:])
        wih_t = const.tile([128, 128], BF16); nc.sync.dma_start(wih_t[:], Wih[:])
        whh_t = const.tile([HID, 128], BF16); nc.sync.dma_start(whh_t[:], Whh[:])
        br_t = const.tile([128, 1], F32); nc.sync.dma_start(br_t[:], br[:])
        wfc_t = const.tile([HID, 1], BF16); nc.sync.dma_start(wfc_t[:], Wfc[:])
        bfc_t = const.tile([1, 1], F32); nc.sync.dma_start(bfc_t[:], bfc[:])
        bw_t = const.tile([128, NW], BF16); nc.sync.dma_start(bw_t[:], Bwarm[:])
        xi_t = const.tile([128, nch], dt.int32); nc.sync.dma_start(xi_t[:], xI[:])
        ohT_t = const.tile([128, nch * 128], FP8); nc.sync.dma_start(ohT_t[:], ohT[:])
        eac_t = const.tile([128, nch], F32); nc.sync.dma_start(eac_t[:], eaC[:])
        dsf_t = const.tile([128, nch], F32); nc.sync.dma_start(dsf_t[:], dstF[:])
        xtd_t = const.tile([F_IN, NB * 128], BF16); nc.sync.dma_start(xtd_t[:], xTD[:])
        meanr = const.tile([128, 1], F32)
        gatT = const.tile([128, NB * 128], BF16)     # [feat, bucket-local node]

        # ---------- Phase 0: mean(edge_attr), local full reduce ----------
        with ExitStack() as ph:
            sbm = ph.enter_context(tc.tile_pool(name="sbm", bufs=1))
            psm = ph.enter_context(tc.tile_pool(name="psm", bufs=1, space="PSUM"))
            eaf_t = sbm.tile([128, EAC], F32)
            nc.sync.dma_start(eaf_t[:], eaF[:])
            eap = sbm.tile([128, 1], F32)
            nc.vector.tensor_reduce(eap[:], eaf_t[:], mybir.AxisListType.X, ALU.add)
            onc = sbm.tile([128, 1], F32)
            nc.gpsimd.memset(onc[:], 1.0)
            ps1 = psm.tile([1, 1], F32, space="PSUM", tag="ps1")
            nc.tensor.matmul(ps1[:], lhsT=eap[:], rhs=onc[:], start=True, stop=True)
            eas = sbm.tile([1, 1], F32)
            nc.scalar.mul(eas[:], ps1[:], 1.0 / E)
            onr = sbm.tile([1, 128], F32)
            nc.gpsimd.memset(onr[:], 1.0)
            ps2 = psm.tile([128, 1], F32, space="PSUM", tag="ps2")
            nc.tensor.matmul(ps2[:], lhsT=onr[:], rhs=eas[:], start=True, stop=True)
            nc.vector.tensor_copy(meanr[:], ps2[:])

        # ---------- Phase 1: self tables (h|a_src|a_dst for own nodes) ----
        sfp = top.enter_context(tc.tile_pool(name="sfp", bufs=1))
        SF = sfp.tile([128, NB * 136], F32)
        SFv = SF[:].rearrange("p (j w) -> p j w", w=136)
        adB = sfp.tile([128, NB * 4], BF16)
        adBv = adB[:].rearrange("p (j w) -> p j w", w=4)
        selfSC = sfp.tile([128, NB * 132], BF16)
        sSCv = selfSC[:].rearrange("p (j w) -> p j w", w=132)
        with ExitStack() as ph:
            sbs = ph.enter_context(tc.tile_pool(name="sbs", bufs=1))
            pss = ph.enter_context(tc.tile_pool(name="pss", bufs=4, space="PSUM"))
            for b_ in range(NB):
                pf = pss.tile([128, 136], F32, space="PSUM", tag="pf")
                nc.tensor.matmul(pf[:], lhsT=xtd_t[:, b_ * 128:(b_ + 1) * 128],
                                 rhs=wall_t[:], start=True, stop=True)
                nc.vector.tensor_copy(SFv[:, b_, :], pf[:])
            nc.vector.tensor_copy(adBv, SFv[:, :, 132:136])
            QS = sbs.tile([128, NB * 4], F32)
            QSv = QS[:].rearrange("p (j w) -> p j w", w=4)
            nc.vector.tensor_tensor(out=QSv, in0=SFv[:, :, 128:132],
                                    in1=SFv[:, :, 132:136], op=ALU.add)
            kapb = kap_t[:].rearrange("p (o w) -> p o w", o=1) \
                .to_broadcast([128, NB, 4])
            nc.vector.scalar_tensor_tensor(out=QSv, in0=kapb, scalar=meanr[:],
                                           op0=ALU.mult, op1=ALU.add, in1=QSv)
            T2 = sbs.tile([128, NB * 4], F32)
            nc.vector.tensor_scalar_mul(T2[:], QS[:], LEAK)
            nc.vector.tensor_tensor(out=QS[:], in0=QS[:], in1=T2[:], op=ALU.max)
            SG1 = sbs.tile([128, NB * 4], F32)
            nc.scalar.activation(SG1[:], QS[:], AF.Sigmoid)
            nc.scalar.activation(T2[:], QS[:], AF.Sigmoid, scale=-1.0)
            nc.vector.reciprocal(T2[:], T2[:])
            SSf = sbs.tile([128, NB * 4], F32)
            nc.vector.tensor_tensor(out=SSf[:], in0=SG1[:], in1=T2[:], op=ALU.mult)
            ssf4 = SSf[:].rearrange("p (j h w) -> p j h w", h=HEADS, w=1) \
                .to_broadcast([128, NB, HEADS, C])
            sf4 = SFv[:, :, 0:128].rearrange("p j (h c) -> p j h c", h=HEADS)
            o4 = sSCv[:, :, 0:128].rearrange("p j (h c) -> p j h c", h=HEADS)
            nc.vector.tensor_tensor(out=o4, in0=sf4, in1=ssf4, op=ALU.mult)
            nc.vector.tensor_copy(sSCv[:, :, 128:132],
                                  SSf[:].rearrange("p (j w) -> p j w", w=4))

        # ---------- Phase 2: edge phase ----------
        NXT = (chb + 1) // 2          # transpose pair count per bucket
        with ExitStack() as ph:
            sbe = ph.enter_context(tc.tile_pool(name="sbe", bufs=2))
            sbq = ph.enter_context(tc.tile_pool(name="sbq", bufs=2))
            pse = ph.enter_context(tc.tile_pool(name="pse", bufs=2, space="PSUM"))
            psh = ph.enter_context(tc.tile_pool(name="psh", bufs=2, space="PSUM"))
            psa = ph.enter_context(tc.tile_pool(name="psa", bufs=2, space="PSUM"))
            psk = ph.enter_context(tc.tile_pool(name="psk", bufs=2, space="PSUM"))
            for b_ in range(NB):
                c0 = b_ * chb
                XGBt = sbe.tile([128, chb * 64], BF16, tag="XGB")
                XGBv = XGBt[:].rearrange("p (e w) -> p e w", w=64)
                for cc in range(chb):
                    nc.gpsimd.indirect_dma_start(
                        out=XGBv[:, cc, :], out_offset=None, in_=xnd[:],
                        in_offset=bass.IndirectOffsetOnAxis(
                            ap=xi_t[:, c0 + cc:c0 + cc + 1], axis=0))
                XGB = XGBt[:]
                XT = sbe.tile([F_IN, chb * 128], BF16, tag="XT")
                for pr in range(NXT):
                    w = min(128, chb * 64 - pr * 128)
                    pxT = psk.tile([128, 128], BF16, space="PSUM", tag="ptr")
                    nc.tensor.transpose(out=pxT[0:w, :],
                                        in_=XGB[:, pr * 128:pr * 128 + w],
                                        identity=identB[:])
                    nc.vector.tensor_copy(XT[:, (2 * pr) * 128:(2 * pr + 1) * 128],
                                          pxT[0:64, :])
                    if w > 64:
                        nc.vector.tensor_copy(
                            XT[:, (2 * pr + 1) * 128:(2 * pr + 2) * 128],
                            pxT[64:128, :])
                # h|a_src per edge; batches of 3 chunks share one PSUM tile
                hEs = sbe.tile([128, chb * 132], BF16, tag="hEs")
                hEv = hEs[:].rearrange("p (e w) -> p e w", w=132)
                Qb = sbq.tile([128, chb * 4], F32, tag="Qb")
                Qv = Qb[:].rearrange("p (e w) -> p e w", w=4)
                nb3 = (chb + 2) // 3
                for b3 in range(nb3):
                    n3 = min(3, chb - b3 * 3)
                    phE = psh.tile([128, 3 * 132], F32, space="PSUM", tag="phE")
                    for j3 in range(n3):
                        cc = b3 * 3 + j3
                        lhs = XT[:, cc * 128:(cc + 1) * 128]
                        nc.tensor.matmul(phE[:, j3 * 132:(j3 + 1) * 132],
                                         lhsT=lhs, rhs=wall_t[:, 0:132],
                                         start=True, stop=True)
                    phEv = phE[:].rearrange("p (e w) -> p e w", w=132)
                    nc.vector.tensor_copy(hEv[:, b3 * 3:b3 * 3 + n3, :],
                                          phEv[:, 0:n3, :])
                    nc.vector.tensor_copy(Qv[:, b3 * 3:b3 * 3 + n3, :],
                                          phEv[:, 0:n3, 128:132])
                # one-hots for the whole bucket in one op
                ohs = sbe.tile([128, chb * 128], FP8, tag="ohs")
                ohv = ohs[:].rearrange("p (e w) -> p e w", w=128)
                dfb = dsf_t[:, c0:c0 + chb].rearrange("p (e w) -> p e w", w=1) \
                    .to_broadcast([128, chb, 128])
                iob = iota_t[:].rearrange("p (o w) -> p o w", o=1) \
                    .to_broadcast([128, chb, 128])
                nc.vector.tensor_tensor(out=ohv, in0=dfb, in1=iob, op=ALU.is_equal)
                # a_dst per edge: one-hot-transpose (host-shipped) matmul
                padc = psa.tile([128, chb * 4], F32, space="PSUM", tag="padc")
                for cc in range(chb):
                    nc.tensor.matmul(
                        padc[:, cc * 4:(cc + 1) * 4],
                        lhsT=ohT_t[:, (c0 + cc) * 128:(c0 + cc + 1) * 128],
                        rhs=adBv[:, b_, :], start=True, stop=True)
                # q = a_src + a_dst + ea*kap ; s = exp(leaky_relu(q))
                kmb = sbq.tile([128, chb * 4], F32, tag="kmb")
                kmv = kmb[:].rearrange("p (e w) -> p e w", w=4)
                eab = eac_t[:, c0:c0 + chb].rearrange("p (e w) -> p e w", w=1) \
                    .to_broadcast([128, chb, 4])
                kab = kap_t[:].rearrange("p (o w) -> p o w", o=1) \
                    .to_broadcast([128, chb, 4])
                nc.vector.tensor_tensor(out=kmv, in0=eab, in1=kab, op=ALU.mult)
                nc.vector.tensor_tensor(out=Qb[:], in0=Qb[:], in1=kmb[:], op=ALU.add)
                nc.vector.tensor_tensor(out=Qb[:], in0=Qb[:], in1=padc[:],
                                        op=ALU.add)
                nc.vector.tensor_scalar_mul(kmb[:], Qb[:], LEAK)
                nc.vector.tensor_tensor(out=Qb[:], in0=Qb[:], in1=kmb[:], op=ALU.max)
                SG = sbq.tile([128, chb * 4], F32, tag="SG")
                nc.scalar.activation(SG[:], Qb[:], AF.Sigmoid)
                nc.scalar.activation(kmb[:], Qb[:], AF.Sigmoid, scale=-1.0)
                nc.vector.reciprocal(kmb[:], kmb[:])
                Sbf = sbq.tile([128, chb * 4], BF16, tag="Sbf")
                nc.vector.tensor_tensor(out=Sbf[:], in0=SG[:], in1=kmb[:], op=ALU.mult)
                Sbv = Sbf[:].rearrange("p (e w) -> p e w", w=4)
                # messages and scatter
                SCb = sbe.tile([128, chb * 132], BF16, tag="SCb")
                SCv = SCb[:].rearrange("p (e w) -> p e w", w=132)
                sb4 = Sbf[:].rearrange("p (e h w) -> p e h w", h=HEADS, w=1) \
                    .to_broadcast([128, chb, HEADS, C])
                he4 = hEv[:, :, 0:128].rearrange("p e (h c) -> p e h c", h=HEADS)
                sc4 = SCv[:, :, 0:128].rearrange("p e (h c) -> p e h c", h=HEADS)
                nc.vector.tensor_tensor(out=sc4, in0=he4, in1=sb4, op=ALU.mult)
                nc.vector.tensor_copy(SCv[:, :, 128:132], Sbv)
                pacc = pse.tile([128, 132], F32, space="PSUM", tag="pacc")
                for cc in range(chb):
                    nc.tensor.matmul(pacc[:], lhsT=ohv[:, cc, :], rhs=SCv[:, cc, :],
                                     start=(cc == 0), stop=(cc == chb - 1))
                # add self loops, normalize, bias, relu, transpose
                nc.vector.tensor_tensor(out=pacc[:], in0=pacc[:],
                                        in1=sSCv[:, b_, :], op=ALU.add)
                dn = sbq.tile([128, 4], F32, tag="dn")
                nc.vector.tensor_scalar_add(dn[:], pacc[:, 128:132], 1e-16)
                nc.vector.reciprocal(dn[:], dn[:])
                gn = sbq.tile([128, 128], F32, tag="gn")
                g4 = gn[:].rearrange("p (h c) -> p h c", h=HEADS)
                p4 = pacc[:, 0:128].rearrange("p (h c) -> p h c", h=HEADS)
                d4 = dn[:].rearrange("p (h w) -> p h w", w=1) \
                    .to_broadcast([128, HEADS, C])
                nc.vector.tensor_tensor(out=g4, in0=p4, in1=d4, op=ALU.mult)
                nc.vector.tensor_tensor(out=gn[:], in0=gn[:], in1=gbr_t[:], op=ALU.add)
                gnb = sbq.tile([128, 128], BF16, tag="gnb")
                nc.vector.tensor_scalar_max(gnb[:], gn[:], 0.0)
                pgT = psk.tile([128, 128], BF16, space="PSUM", tag="ptr")
                nc.tensor.transpose(out=pgT[:], in_=gnb[:], identity=identB[:])
                nc.vector.tensor_copy(gatT[:, b_ * 128:(b_ + 1) * 128], pgT[:])

        # ---------- Phase 3: gx = Wih.T @ gatT (+ warmup mask), bf16 ----
        persist = top.enter_context(tc.tile_pool(name="persist", bufs=1))
        Gxb = persist.tile([128, NTL], BF16)
        H = persist.tile([HID, NTL], BF16)
        nc.gpsimd.memset(H[:], 0.0)
        GSL = [(0, 512), (512, 1024), (1024, NTL)]
        with ExitStack() as ph:
            psg = ph.enter_context(tc.tile_pool(name="psg", bufs=3, space="PSUM"))
            for lo, hi in GSL:
                pgx = psg.tile([128, 512], F32, space="PSUM", tag="pgx")
                nc.tensor.matmul(pgx[:, 0:hi - lo], lhsT=wih_t[:],
                                 rhs=gatT[:, 32 + lo:32 + hi],
                                 start=True, stop=True)
                nc.vector.tensor_copy(Gxb[:, lo:hi], pgx[:, 0:hi - lo])
            nc.vector.tensor_tensor(out=Gxb[:, 0:NW], in0=Gxb[:, 0:NW],
                                    in1=bw_t[:], op=ALU.add)

        # ---------- Phase 4: LSTM fixed point ----------
        # One chained [32, NTL] sequence; warmup absorbs the halo boundary.
        # gx lands in PSUM via a PE identity matmul so the Whh matmul can
        # accumulate onto it within a normal PE accumulation group.
        YS = persist.tile([HID, SEQ], BF16)
        HB = 688                     # column-half boundary for pipelining
        GSLH = [[(0, 512), (512, HB)], [(HB, 1024), (1024, NTL)]]
        with ExitStack() as ph:
            sbl = ph.enter_context(tc.tile_pool(name="sbl", bufs=2))
            psl = ph.enter_context(tc.tile_pool(name="psl", bufs=2, space="PSUM"))
            for it in range(ITERS):
                pG = psl.tile([128, 2048], F32, space="PSUM", tag="pG")
                S_ = sbl.tile([96, NTL], BF16, tag="S")
                Tg = sbl.tile([64, NTL], BF16, tag="Tg")
                Zt = sbl.tile([HID, NTL], BF16, tag="Zt")
                Ct = sbl.tile([HID, NTL], BF16, tag="Ct")
                TCu = sbl.tile([96, NTL], BF16, tag="TCu")
                for hf in range(2):
                    h0 = hf * HB
                    h1 = HB if hf == 0 else NTL
                    for lo, hi in GSLH[hf]:
                        nc.tensor.matmul(pG[:, lo:hi], lhsT=identB[:],
                                         rhs=Gxb[:, lo:hi], start=True,
                                         stop=(it == 0))
                    if it > 0:
                        for lo, hi in GSLH[hf]:
                            nc.tensor.matmul(pG[:, lo:hi], lhsT=whh_t[:],
                                             rhs=H[:, lo:hi], start=False,
                                             stop=True)
                    nc.scalar.activation(S_[:, h0:h1], pG[0:96, h0:h1],
                                         AF.Sigmoid, bias=br_t[0:96, :])
                    nc.scalar.activation(Tg[32:64, h0:h1], pG[96:128, h0:h1],
                                         AF.Tanh, bias=br_t[96:128, :])
                    nc.vector.tensor_tensor(out=Zt[:, h0:h1],
                                            in0=S_[32:64, h0:h1],
                                            in1=Tg[32:64, h0:h1], op=ALU.mult)
                    nc.vector.tensor_tensor_scan(
                        out=Ct[:, h0:h1], data0=S_[0:32, h0:h1],
                        data1=Zt[:, h0:h1],
                        initial=(0.0 if hf == 0 else Ct[:, HB - 1:HB]),
                        op0=ALU.mult, op1=ALU.add)
                    nc.scalar.activation(TCu[64:96, h0:h1], Ct[:, h0:h1],
                                         AF.Tanh)
                    if it < ITERS - 1:
                        # h feeds gates one step later; the last column of
                        # the sequence is never consumed
                        he = h1 + 1 if hf == 0 else h1
                        nc.gpsimd.tensor_tensor(
                            out=H[0:32, h0 + 1:he],
                            in0=S_[64:96, h0:he - 1],
                            in1=TCu[64:96, h0:he - 1], op=ALU.mult)
                    else:
                        y0 = 0 if hf == 0 else HB - NW
                        nc.vector.tensor_tensor(
                            out=YS[:, y0:h1 - NW],
                            in0=S_[64:96, max(h0, NW):h1],
                            in1=TCu[64:96, max(h0, NW):h1], op=ALU.mult)

        # ---------- Phase 5: FC ----------
        with ExitStack() as ph:
            sbf = ph.enter_context(tc.tile_pool(name="sbf", bufs=1))
            psf = ph.enter_context(tc.tile_pool(name="psf", bufs=4, space="PSUM"))
            OF = sbf.tile([1, SEQ], F32)
            for q in range(4):
                pf = psf.tile([1, 320], F32, space="PSUM", tag="pfc")
                nc.tensor.matmul(pf[:], lhsT=wfc_t[:],
                                 rhs=YS[:, q * 320:(q + 1) * 320],
                                 start=True, stop=True)
                nc.vector.tensor_scalar_add(OF[:, q * 320:(q + 1) * 320],
                                            pf[:], bfc_t[:])
            nc.sync.dma_start(out[:], OF[:])

    nc.compile()
    return nc


def run(inputs, trace=False):
    in_maps, chb = _prep_host(inputs)
    if chb not in _CACHE:
        _CACHE[chb] = _build_nc(chb)
    nc = _CACHE[chb]
    res = run_bass_kernel_spmd(nc, in_maps, list(range(NCORES)), trace=trace)
    return res


def kernel(**inputs) -> np.ndarray:
    res = run(inputs)
    full = np.concatenate([np.asarray(res.results[k]["out"][0], np.float32)
                           for k in range(NCORES)])
    return np.ascontiguousarray(full[:N].reshape(N, 1))
